# revision 8
# baseline (speedup 1.0000x reference)
"""Trainium2 Bass kernel for Tacotron2-style location-sensitive attention.

Computation (per batch b):
    pq   = tanh(hidden @ W_query)                      [128]
    pm   = tanh(memory @ W_memory)                     [T, 128]
    conv = conv1d_same(attention_weights_cat.T)        [T, 32]
    ploc = tanh(conv @ W_loc)                          [T, 128]
    e    = tanh(pq + ploc + pm) @ W_v                  [T]
    w    = softmax(e)                                  [T]
    ctx  = w @ memory                                  [512]

Sharding: data-parallel, batch 64 -> 8 cores x 8 batches. Weights replicated.

Device layout choices:
  - memory is shipped twice in bf16 (same total bytes as fp32 once):
      memT [512, T]  (m on partitions) -> feeds the pm matmul (contract m)
      memN [T, 512]  (t on partitions) -> feeds the context matmul (contract t)
  - conv is an im2col matmul: host builds 62 shifted copies of the 2 input
    rows; one K=62 matmul per t-slice.
  - all elementwise math runs in [d=128, t] layout so pq is a per-partition
    activation bias; energies come out as [t%128, t//128] so the softmax
    reductions run on 128 lanes.
  - softmax skips max-subtraction: |e| <= ||W_v||_1 ~ 11, exp() is safe in
    fp32 and the result is identical.
"""

import numpy as np
import ml_dtypes

BF16 = ml_dtypes.bfloat16
B, T = 64, 2000
Q_DIM, MEM_DIM, ATT_DIM = 1024, 512, 128
N_FILTERS, KERNEL = 32, 31
N_CORES = 8
BL = B // N_CORES            # local batch per core
TS = 500                     # t-slice width for 128xTS matmuls (4 slices)
NTS = T // TS
NC_CH = 16                   # t chunks of 128 for energies/context (15*128+80)
TAIL = T - 15 * 128          # 80


def _split_excess_waits(nc, limit=1):
    """This container's walrus build rejects >`limit` semaphore waits per
    instruction. Move excess waits onto preceding same-engine NoOps (same
    semantics: per-engine program order, each wait blocks the sequencer)."""
    import bass_rust
    ctr = 0
    for fn in nc.m.functions:
        for blk in fn.blocks:
            new = []
            changed = False
            for inst in blk.instructions:
                si = inst.sync_info
                if si is not None and si.on_wait and len(si.on_wait) > limit:
                    waits = list(si.on_wait)
                    n_excess = len(waits) - limit
                    for i in range(0, n_excess, limit):
                        n = bass_rust.InstNoOp(
                            name=f"I-wsplit-{ctr}", engine=inst.engine)
                        ctr += 1
                        n.sync_info = bass_rust.SyncInfo(
                            on_wait=waits[i:i + limit], on_update=[])
                        new.append(n)
                    si.on_wait = waits[n_excess:]
                    changed = True
                new.append(inst)
            if changed:
                blk.instructions = new
    return ctr


_NC_CACHE = {}


def _build_bass():
    if "nc" in _NC_CACHE:
        return _NC_CACHE["nc"]
    import concourse.bass as bass
    import concourse.tile as tile
    from concourse import mybir

    f32 = mybir.dt.float32
    bf16 = mybir.dt.bfloat16
    Tanh = mybir.ActivationFunctionType.Tanh
    Exp = mybir.ActivationFunctionType.Exp

    nc = bass.Bass()

    memT = nc.dram_tensor("memT", [BL, MEM_DIM, T], bf16, kind="ExternalInput")
    memN = nc.dram_tensor("memN", [BL, NC_CH * 128, MEM_DIM], bf16,
                          kind="ExternalInput")
    win = nc.dram_tensor("win", [BL, 2 * KERNEL, T], bf16,
                         kind="ExternalInput")
    hT = nc.dram_tensor("hT", [Q_DIM // 128, 128, BL], f32,
                        kind="ExternalInput")
    wq = nc.dram_tensor("wq", [Q_DIM // 128, 128, ATT_DIM], f32,
                        kind="ExternalInput")
    wm = nc.dram_tensor("wm", [MEM_DIM // 128, 128, ATT_DIM], bf16,
                        kind="ExternalInput")
    wconv = nc.dram_tensor("wconv", [2 * KERNEL, N_FILTERS], bf16,
                           kind="ExternalInput")
    wloc = nc.dram_tensor("wloc", [N_FILTERS, ATT_DIM], bf16,
                          kind="ExternalInput")
    wv = nc.dram_tensor("wv", [ATT_DIM, 1], bf16, kind="ExternalInput")

    out_ctx = nc.dram_tensor("out_ctx", [BL, MEM_DIM], f32,
                             kind="ExternalOutput")
    out_w = nc.dram_tensor("out_w", [BL, T], f32, kind="ExternalOutput")

    with tile.TileContext(nc) as tc:
        with (
            tc.tile_pool(name="consts", bufs=1) as consts,
            tc.tile_pool(name="big", bufs=2) as big,
            tc.tile_pool(name="mid", bufs=2) as mid,
            tc.tile_pool(name="small", bufs=2) as small,
            tc.tile_pool(name="ps_pm", bufs=2, space="PSUM") as ps_pm,
            tc.tile_pool(name="ps_cv", bufs=1, space="PSUM") as ps_cv,
            tc.tile_pool(name="ps_pl", bufs=1, space="PSUM") as ps_pl,
            tc.tile_pool(name="ps_e", bufs=1, space="PSUM") as ps_e,
            tc.tile_pool(name="ps_sm", bufs=1, space="PSUM") as ps_sm,
            tc.tile_pool(name="ps_cx", bufs=1, space="PSUM") as ps_cx,
        ):
            # ---- constants ----
            wq_sb = consts.tile([128, Q_DIM // 128, ATT_DIM], f32)
            nc.sync.dma_start(out=wq_sb, in_=wq.rearrange("k p d -> p k d"))
            wm_sb = consts.tile([128, MEM_DIM // 128, ATT_DIM], bf16)
            nc.sync.dma_start(out=wm_sb, in_=wm.rearrange("k p d -> p k d"))
            wconv_sb = consts.tile([2 * KERNEL, N_FILTERS], bf16)
            nc.sync.dma_start(out=wconv_sb, in_=wconv[:, :])
            wloc_sb = consts.tile([N_FILTERS, ATT_DIM], bf16)
            nc.sync.dma_start(out=wloc_sb, in_=wloc[:, :])
            wv_sb = consts.tile([ATT_DIM, 1], bf16)
            nc.sync.dma_start(out=wv_sb, in_=wv[:, :])
            hT_sb = consts.tile([128, Q_DIM // 128, BL], f32)
            nc.sync.dma_start(out=hT_sb, in_=hT.rearrange("k p b -> p k b"))
            ones_col = consts.tile([128, 1], f32)
            nc.vector.memset(ones_col, 1.0)
            ones_row = consts.tile([1, 128], f32)
            nc.vector.memset(ones_row, 1.0)

            # ---- pq for all local batches: [d=128, b=BL] ----
            pq_ps = ps_sm.tile([128, BL], f32, tag="bc")
            for kq in range(Q_DIM // 128):
                nc.tensor.matmul(pq_ps, lhsT=wq_sb[:, kq, :],
                                 rhs=hT_sb[:, kq, :],
                                 start=(kq == 0), stop=(kq == Q_DIM // 128 - 1))
            pqT = consts.tile([128, BL], f32)
            nc.scalar.activation(out=pqT, in_=pq_ps, func=Tanh)

            for b in range(BL):
                # ---- loads ----
                mT = big.tile([128, MEM_DIM // 128, T], bf16, tag="mT")
                nc.sync.dma_start(
                    out=mT, in_=memT[b].rearrange("(k p) t -> p k t", p=128))
                nat = big.tile([128, NC_CH, MEM_DIM], bf16, tag="nat")
                nc.sync.dma_start(
                    out=nat, in_=memN[b].rearrange("(c p) m -> p c m", p=128))
                wint = mid.tile([2 * KERNEL, T], bf16, tag="win")
                nc.sync.dma_start(out=wint, in_=win[b])

                tpm = mid.tile([128, T], bf16, tag="tpm")
                cvt = mid.tile([N_FILTERS, T], bf16, tag="cvt")
                s12 = mid.tile([128, T], bf16, tag="s12")
                inner = mid.tile([128, T], bf16, tag="inner")

                for ts in range(NTS):
                    sl = slice(ts * TS, (ts + 1) * TS)
                    # pm = memory @ W_memory  (contract m, 4 k-chunks)
                    pm_ps = ps_pm.tile([128, TS], f32, tag="pm")
                    for k in range(MEM_DIM // 128):
                        nc.tensor.matmul(pm_ps, lhsT=wm_sb[:, k, :],
                                         rhs=mT[:, k, sl],
                                         start=(k == 0),
                                         stop=(k == MEM_DIM // 128 - 1))
                    nc.scalar.activation(out=tpm[:, sl], in_=pm_ps, func=Tanh)

                    # conv (im2col K=62 matmul)
                    cv_ps = ps_cv.tile([N_FILTERS, TS], f32, tag="cv")
                    nc.tensor.matmul(cv_ps, lhsT=wconv_sb, rhs=wint[:, sl],
                                     start=True, stop=True)
                    nc.vector.tensor_copy(out=cvt[:, sl], in_=cv_ps)

                    # ploc = conv @ W_loc (K=32)
                    pl_ps = ps_pl.tile([128, TS], f32, tag="pl")
                    nc.tensor.matmul(pl_ps, lhsT=wloc_sb, rhs=cvt[:, sl],
                                     start=True, stop=True)
                    # s12 = tanh(pm) + tanh(ploc); inner = tanh(s12 + pq)
                    nc.scalar.activation(out=s12[:, sl], in_=pl_ps, func=Tanh)
                    nc.vector.tensor_add(out=s12[:, sl], in0=s12[:, sl],
                                         in1=tpm[:, sl])
                    nc.scalar.activation(out=inner[:, sl], in_=s12[:, sl],
                                         func=Tanh, bias=pqT[:, b:b + 1])

                # ---- energies: [t%128, t//128] ----
                e_ps = ps_e.tile([128, NC_CH], f32, tag="e")
                # pre-fill last chunk column; matmul overwrites rows < TAIL,
                # rows >= TAIL keep -1e30 so exp() gives exactly 0 there
                nc.vector.memset(e_ps[:, NC_CH - 1:NC_CH], -1e30)
                for c in range(NC_CH):
                    width = 128 if c < NC_CH - 1 else TAIL
                    nc.tensor.matmul(
                        e_ps[0:width, c:c + 1],
                        lhsT=inner[:, c * 128:c * 128 + width],
                        rhs=wv_sb, start=True, stop=True)

                # ---- softmax (no max-subtraction; |e| <= ~11) ----
                exps = small.tile([128, NC_CH], f32, tag="exp")
                rowsum = small.tile([128, 1], f32, tag="rs")
                nc.scalar.activation(out=exps, in_=e_ps, func=Exp,
                                     accum_out=rowsum)
                tot_ps = ps_sm.tile([1, 1], f32, tag="bc")
                nc.tensor.matmul(tot_ps, lhsT=rowsum, rhs=ones_col,
                                 start=True, stop=True)
                rec = small.tile([1, 1], f32, tag="rec")
                nc.vector.reciprocal(out=rec, in_=tot_ps)
                bc_ps = ps_sm.tile([128, 1], f32, tag="bc")
                nc.tensor.matmul(bc_ps, lhsT=ones_row, rhs=rec,
                                 start=True, stop=True)
                bsb = small.tile([128, 1], f32, tag="bsb")
                nc.vector.tensor_copy(out=bsb, in_=bc_ps)
                wsb = small.tile([128, NC_CH], f32, tag="wsb")
                nc.vector.tensor_scalar_mul(out=wsb, in0=exps, scalar1=bsb)
                wbf = small.tile([128, NC_CH], bf16, tag="wbf")
                nc.vector.tensor_scalar_mul(out=wbf, in0=exps, scalar1=bsb)

                # ---- context: w @ memory (contract t, 16 chunks) ----
                cx_ps = ps_cx.tile([1, MEM_DIM], f32, tag="cx")
                for c in range(NC_CH):
                    nc.tensor.matmul(cx_ps, lhsT=wbf[:, c:c + 1],
                                     rhs=nat[:, c, :],
                                     start=(c == 0), stop=(c == NC_CH - 1))
                csb = small.tile([1, MEM_DIM], f32, tag="csb")
                nc.vector.tensor_copy(out=csb, in_=cx_ps)

                # ---- outputs ----
                nc.sync.dma_start(out=out_ctx[b:b + 1, :], in_=csb)
                nc.sync.dma_start(
                    out=out_w[b, 0:15 * 128].rearrange("(c p) -> p c", p=128),
                    in_=wsb[:, 0:15])
                nc.sync.dma_start(
                    out=out_w[b, 15 * 128:T].rearrange("(p c) -> p c", c=1),
                    in_=wsb[0:TAIL, 15:16])

    _split_excess_waits(nc)
    _NC_CACHE["nc"] = nc
    return nc


def _host_prep(attention_hidden_state, memory, attention_weights_cat,
               W_query, W_memory, W_v, W_conv, W_loc):
    """Pure layout transforms (transpose/cast/pad/im2col) — no model math."""
    mem_bf = memory.astype(BF16)                       # [B, T, 512]
    memT = np.ascontiguousarray(mem_bf.transpose(0, 2, 1))   # [B, 512, T]
    memN = np.zeros((B, NC_CH * 128, MEM_DIM), BF16)
    memN[:, :T] = mem_bf

    win = np.zeros((B, 2 * KERNEL, T), np.float32)
    awc = np.asarray(attention_weights_cat)
    for k in range(KERNEL):
        sh = k - (KERNEL // 2)
        lo, hi = max(0, -sh), min(T, T - sh)
        win[:, 2 * k, lo:hi] = awc[:, 0, lo + sh:hi + sh]
        win[:, 2 * k + 1, lo:hi] = awc[:, 1, lo + sh:hi + sh]
    win = win.astype(BF16)

    hTd = np.ascontiguousarray(attention_hidden_state.T)     # [1024, B]
    per_core = []
    for i in range(N_CORES):
        bs = slice(i * BL, (i + 1) * BL)
        per_core.append({
            "memT": np.ascontiguousarray(memT[bs]),
            "memN": np.ascontiguousarray(memN[bs]),
            "win": np.ascontiguousarray(win[bs]),
            "hT": np.ascontiguousarray(
                hTd[:, bs].reshape(Q_DIM // 128, 128, BL)),
            "wq": np.ascontiguousarray(
                W_query.reshape(Q_DIM // 128, 128, ATT_DIM)).astype(np.float32),
            "wm": np.ascontiguousarray(
                W_memory.reshape(MEM_DIM // 128, 128, ATT_DIM)).astype(BF16),
            "wconv": np.ascontiguousarray(
                W_conv.reshape(2 * KERNEL, N_FILTERS)).astype(BF16),
            "wloc": np.ascontiguousarray(W_loc).astype(BF16),
            "wv": np.ascontiguousarray(W_v).astype(BF16),
        })
    return per_core


def kernel(attention_hidden_state, memory, attention_weights_cat,
           W_query, W_memory, W_v, W_conv, W_loc):
    from concourse.bass_utils import run_bass_kernel_spmd

    args = dict(
        attention_hidden_state=np.asarray(attention_hidden_state, np.float32),
        memory=np.asarray(memory, np.float32),
        attention_weights_cat=np.asarray(attention_weights_cat, np.float32),
        W_query=np.asarray(W_query, np.float32),
        W_memory=np.asarray(W_memory, np.float32),
        W_v=np.asarray(W_v, np.float32),
        W_conv=np.asarray(W_conv, np.float32),
        W_loc=np.asarray(W_loc, np.float32),
    )
    in_maps = _host_prep(**args)
    nc = _build_bass()
    res = run_bass_kernel_spmd(nc, in_maps, core_ids=list(range(N_CORES)))
    ctx = np.concatenate([r["out_ctx"] for r in res.results], axis=0)
    w = np.concatenate([r["out_w"] for r in res.results], axis=0)
    return ctx, w


# revision 27
# speedup vs baseline: 1.2280x; 1.2280x over previous
"""Trainium2 Bass kernel for Tacotron2-style location-sensitive attention.

Computation (per batch b):
    pq   = tanh(hidden @ W_query)                      [128]
    pm   = tanh(memory @ W_memory)                     [T, 128]
    conv = conv1d_same(attention_weights_cat.T)        [T, 32]
    ploc = tanh(conv @ W_loc)                          [T, 128]
    e    = tanh(pq + ploc + pm) @ W_v                  [T]
    w    = softmax(e)                                  [T]
    ctx  = w @ memory                                  [512]

Sharding: data-parallel, batch 64 -> 8 cores x 8 batches. Weights replicated.

Device layout choices:
  - memory is shipped twice in bf16 (same total bytes as fp32 once):
      memT [512, T]  (m on partitions) -> feeds the pm matmul (contract m)
      memN [T, 512]  (t on partitions) -> feeds the context matmul (contract t)
  - conv is an im2col matmul: host builds 62 shifted copies of the 2 input
    rows (packed 124 partitions x 2 t-halves); one K=62 matmul per t-slice.
  - all elementwise math runs in [d=128, t] layout so pq is a per-partition
    activation bias; energies come out as [t%128, t//128] so the softmax
    reductions run on 128 lanes.
  - softmax skips max-subtraction: |e| <= ||W_v||_1 ~ 11, exp() is safe in
    fp32 and the result is identical.
  - two-phase software pipeline: phase 1 streams memT+win and computes
    pm/conv/ploc/inner per batch; phase 2 streams memN and runs
    energies/softmax/context as chunks arrive, so the DMA engines (the
    bottleneck resource) stay busy end to end.
"""

import numpy as np
import ml_dtypes

BF16 = ml_dtypes.bfloat16
B, T = 64, 2000
Q_DIM, MEM_DIM, ATT_DIM = 1024, 512, 128
N_FILTERS, KERNEL = 32, 31
N_CORES = 8
BL = B // N_CORES            # local batch per core
TS = 500                     # t-slice width for 128xTS matmuls (4 slices)
NTS = T // TS
NC_CH = 16                   # t chunks of 128 for energies/context (15*128+80)
TAIL = T - 15 * 128          # 80
NWIN = 2 * KERNEL            # 62 im2col rows


def _split_excess_waits(nc, limit=1):
    """This container's walrus build rejects >`limit` semaphore waits per
    instruction. Move excess waits onto preceding same-engine NoOps (same
    semantics: per-engine program order, each wait blocks the sequencer)."""
    import bass_rust
    ctr = 0
    for fn in nc.m.functions:
        for blk in fn.blocks:
            new = []
            changed = False
            for inst in blk.instructions:
                si = inst.sync_info
                if si is not None and si.on_wait and len(si.on_wait) > limit:
                    waits = list(si.on_wait)
                    n_excess = len(waits) - limit
                    for i in range(0, n_excess, limit):
                        n = bass_rust.InstNoOp(
                            name=f"I-wsplit-{ctr}", engine=inst.engine)
                        ctr += 1
                        n.sync_info = bass_rust.SyncInfo(
                            on_wait=waits[i:i + limit], on_update=[])
                        new.append(n)
                    si.on_wait = waits[n_excess:]
                    changed = True
                new.append(inst)
            if changed:
                blk.instructions = new
    return ctr


_NC_CACHE = {}


def _build_bass():
    if "nc" in _NC_CACHE:
        return _NC_CACHE["nc"]
    import concourse.bass as bass
    import concourse.tile as tile
    from concourse import mybir

    f32 = mybir.dt.float32
    bf16 = mybir.dt.bfloat16
    Tanh = mybir.ActivationFunctionType.Tanh
    Exp = mybir.ActivationFunctionType.Exp

    nc = bass.Bass()

    memT = nc.dram_tensor("memT", [BL, MEM_DIM, T], bf16, kind="ExternalInput")
    memN = nc.dram_tensor("memN", [BL, NC_CH * 128, MEM_DIM], bf16,
                          kind="ExternalInput")
    win = nc.dram_tensor("win", [BL, NWIN, T], bf16,
                         kind="ExternalInput")
    hT = nc.dram_tensor("hT", [Q_DIM // 128, 128, BL], f32,
                        kind="ExternalInput")
    wq = nc.dram_tensor("wq", [Q_DIM // 128, 128, ATT_DIM], f32,
                        kind="ExternalInput")
    wm = nc.dram_tensor("wm", [MEM_DIM // 128, 128, ATT_DIM], bf16,
                        kind="ExternalInput")
    wconv = nc.dram_tensor("wconv", [NWIN, N_FILTERS], bf16,
                           kind="ExternalInput")
    wloc = nc.dram_tensor("wloc", [N_FILTERS, ATT_DIM], bf16,
                          kind="ExternalInput")
    wv = nc.dram_tensor("wv", [ATT_DIM, 1], bf16, kind="ExternalInput")

    out_ctx = nc.dram_tensor("out_ctx", [BL, MEM_DIM], f32,
                             kind="ExternalOutput")
    out_w = nc.dram_tensor("out_w", [BL, T], f32, kind="ExternalOutput")

    with tile.TileContext(nc) as tc:
        with (
            tc.tile_pool(name="consts", bufs=1) as consts,
            tc.tile_pool(name="big", bufs=3) as big,
            tc.tile_pool(name="natp", bufs=3) as natp,
            tc.tile_pool(name="innp", bufs=8) as innp,
            tc.tile_pool(name="mid", bufs=3) as mid,
            tc.tile_pool(name="small", bufs=3) as small,
            tc.tile_pool(name="ps_pm", bufs=3, space="PSUM") as ps_pm,
            tc.tile_pool(name="ps_cv", bufs=1, space="PSUM") as ps_cv,
            tc.tile_pool(name="ps_pl", bufs=1, space="PSUM") as ps_pl,
            tc.tile_pool(name="ps_e", bufs=1, space="PSUM") as ps_e,
            tc.tile_pool(name="ps_sm", bufs=1, space="PSUM") as ps_sm,
            tc.tile_pool(name="ps_cx", bufs=1, space="PSUM") as ps_cx,
        ):
            # first big load goes out before the (small) weight loads
            mT0 = big.tile([128, MEM_DIM // 128, T], bf16, tag="mT")
            nc.sync.dma_start(
                out=mT0, in_=memT[0].rearrange("(k p) t -> p k t", p=128))

            # ---- constants ----
            wq_sb = consts.tile([128, Q_DIM // 128, ATT_DIM], f32)
            nc.sync.dma_start(out=wq_sb, in_=wq.rearrange("k p d -> p k d"))
            wm_sb = consts.tile([128, MEM_DIM // 128, ATT_DIM], bf16)
            nc.sync.dma_start(out=wm_sb, in_=wm.rearrange("k p d -> p k d"))
            wconv_sb = consts.tile([NWIN, N_FILTERS], bf16)
            nc.sync.dma_start(out=wconv_sb, in_=wconv[:, :])
            wloc_sb = consts.tile([N_FILTERS, ATT_DIM], bf16)
            nc.sync.dma_start(out=wloc_sb, in_=wloc[:, :])
            wv_sb = consts.tile([ATT_DIM, 1], bf16)
            nc.sync.dma_start(out=wv_sb, in_=wv[:, :])
            hT_sb = consts.tile([128, Q_DIM // 128, BL], f32)
            nc.sync.dma_start(out=hT_sb, in_=hT.rearrange("k p b -> p k b"))
            ones_col = consts.tile([128, 1], f32)
            nc.vector.memset(ones_col, 1.0)
            ones_row = consts.tile([1, 128], f32)
            nc.vector.memset(ones_row, 1.0)

            # ---- pq for all local batches: [d=128, b=BL] ----
            pq_ps = ps_sm.tile([128, BL], f32, tag="bc")
            for kq in range(Q_DIM // 128):
                nc.tensor.matmul(pq_ps, lhsT=wq_sb[:, kq, :],
                                 rhs=hT_sb[:, kq, :],
                                 start=(kq == 0), stop=(kq == Q_DIM // 128 - 1))
            pqT = consts.tile([128, BL], f32)
            nc.scalar.activation(out=pqT, in_=pq_ps, func=Tanh)

            state = {}
            nat_tiles = {}

            def nat_load(b):
                # memN streamed in 4 chunk-groups so the context matmuls can
                # start on early chunks while later ones are still in flight
                nat = natp.tile([128, NC_CH, MEM_DIM], bf16, tag="nat")
                natsrc = memN[b].rearrange("(c p) m -> p c m", p=128)
                for g in range(4):
                    nc.sync.dma_start(out=nat[:, 4 * g:4 * g + 4, :],
                                      in_=natsrc[:, 4 * g:4 * g + 4, :])
                nat_tiles[b] = nat

            def head(b):
                if b == 0:
                    mT = mT0
                else:
                    mT = big.tile([128, MEM_DIM // 128, T], bf16, tag="mT")
                    nc.sync.dma_start(
                        out=mT,
                        in_=memT[b].rearrange("(k p) t -> p k t", p=128))
                wint = mid.tile([NWIN, T], bf16, tag="win")
                nc.sync.dma_start(out=wint, in_=win[b])
                if b >= BL - 3:
                    nat_load(b - (BL - 3))      # prefetch nat(0..2)

                tpm = mid.tile([128, T], bf16, tag="tpm")
                cvt = mid.tile([N_FILTERS, T], bf16, tag="cvt")
                s12 = mid.tile([128, T], bf16, tag="s12")
                inner = innp.tile([128, T], bf16, tag="inner")

                for ts in range(NTS):
                    sl = slice(ts * TS, (ts + 1) * TS)
                    # pm = memory @ W_memory  (contract m, 4 k-chunks)
                    pm_ps = ps_pm.tile([128, TS], f32, tag="pm")
                    for k in range(MEM_DIM // 128):
                        nc.tensor.matmul(pm_ps, lhsT=wm_sb[:, k, :],
                                         rhs=mT[:, k, sl],
                                         start=(k == 0),
                                         stop=(k == MEM_DIM // 128 - 1))
                    nc.scalar.activation(out=tpm[:, sl], in_=pm_ps, func=Tanh)

                    # conv (im2col K=62 matmul); wint holds two t-halves
                    # stacked on partitions: rows [62h, 62h+62) = t-half h
                    cv_ps = ps_cv.tile([N_FILTERS, TS], f32, tag="cv")
                    nc.tensor.matmul(cv_ps, lhsT=wconv_sb, rhs=wint[:, sl],
                                     start=True, stop=True)
                    nc.vector.tensor_copy(out=cvt[:, sl], in_=cv_ps)

                    # ploc = conv @ W_loc (K=32)
                    pl_ps = ps_pl.tile([128, TS], f32, tag="pl")
                    nc.tensor.matmul(pl_ps, lhsT=wloc_sb, rhs=cvt[:, sl],
                                     start=True, stop=True)
                    # s12 = tanh(pm) + tanh(ploc); inner = tanh(s12 + pq)
                    nc.scalar.activation(out=s12[:, sl], in_=pl_ps, func=Tanh)
                    nc.vector.tensor_add(out=s12[:, sl], in0=s12[:, sl],
                                         in1=tpm[:, sl])
                    nc.scalar.activation(out=inner[:, sl], in_=s12[:, sl],
                                         func=Tanh, bias=pqT[:, b:b + 1])
                state[b] = inner

            def tail(b):
                inner = state.pop(b)
                if b + 3 < BL:
                    nat_load(b + 3)
                nat = nat_tiles.pop(b)

                # ---- energies: [t%128, t//128] ----
                e_ps = ps_e.tile([128, NC_CH], f32, tag="e")
                # pre-fill last chunk column; matmul overwrites rows < TAIL,
                # rows >= TAIL keep -1e30 so exp() gives exactly 0 there
                nc.vector.memset(e_ps[:, NC_CH - 1:NC_CH], -1e30)
                for c in range(NC_CH):
                    width = 128 if c < NC_CH - 1 else TAIL
                    nc.tensor.matmul(
                        e_ps[0:width, c:c + 1],
                        lhsT=inner[:, c * 128:c * 128 + width],
                        rhs=wv_sb, start=True, stop=True)

                # ---- softmax (no max-subtraction; |e| <= ~11) ----
                exps = small.tile([128, NC_CH], f32, tag="exp")
                rowsum = small.tile([128, 1], f32, tag="rs")
                nc.scalar.activation(out=exps, in_=e_ps, func=Exp,
                                     accum_out=rowsum)
                tot_ps = ps_sm.tile([1, 1], f32, tag="bc")
                nc.tensor.matmul(tot_ps, lhsT=rowsum, rhs=ones_col,
                                 start=True, stop=True)
                rec = small.tile([1, 1], f32, tag="rec")
                nc.vector.reciprocal(out=rec, in_=tot_ps)
                bc_ps = ps_sm.tile([128, 1], f32, tag="bc")
                nc.tensor.matmul(bc_ps, lhsT=ones_row, rhs=rec,
                                 start=True, stop=True)
                bsb = small.tile([128, 1], f32, tag="bsb")
                nc.vector.tensor_copy(out=bsb, in_=bc_ps)
                wsb = small.tile([128, NC_CH], f32, tag="wsb")
                nc.vector.tensor_scalar_mul(out=wsb, in0=exps, scalar1=bsb)

                # normalized weights, placed in column b of a zeroed
                # [128, NC_CH, BL] operand: the context matmul then lands
                # batch b in PSUM row b of ONE shared [BL, 512] accumulator
                # (all 8 batches -> a single efficient output DMA).
                wbf = small.tile([128, NC_CH, BL], bf16, tag="wbf")
                nc.vector.memset(wbf, 0.0)
                nc.vector.tensor_scalar_mul(out=wbf[:, :, b], in0=exps,
                                            scalar1=bsb)
                for c in range(NC_CH):
                    nc.tensor.matmul(cx8_ps, lhsT=wbf[:, c, :],
                                     rhs=nat[:, c, :],
                                     start=(b == 0 and c == 0),
                                     stop=(b == BL - 1 and c == NC_CH - 1),
                                     skip_group_check=True)

                # ---- weight outputs (SWDGE so the SP never blocks) ----
                nc.gpsimd.dma_start(
                    out=out_w[b, 0:15 * 128].rearrange("(c p) -> p c", p=128),
                    in_=wsb[:, 0:15])
                nc.gpsimd.dma_start(
                    out=out_w[b, 15 * 128:T].rearrange("(p c) -> p c", c=1),
                    in_=wsb[0:TAIL, 15:16])

            cx8_ps = ps_cx.tile([BL, MEM_DIM], f32, tag="cx")
            for b in range(BL):
                head(b)
            for b in range(BL):
                tail(b)
            ctx8 = small.tile([BL, MEM_DIM], f32, tag="ctx8")
            nc.vector.tensor_copy(out=ctx8, in_=cx8_ps)
            nc.sync.dma_start(out=out_ctx[:, :], in_=ctx8)

    _split_excess_waits(nc)
    _NC_CACHE["nc"] = nc
    return nc


def _host_prep(attention_hidden_state, memory, attention_weights_cat,
               W_query, W_memory, W_v, W_conv, W_loc):
    """Pure layout transforms (transpose/cast/pad/im2col) — no model math."""
    mem_bf = memory.astype(BF16)                       # [B, T, 512]
    memT = np.ascontiguousarray(mem_bf.transpose(0, 2, 1))   # [B, 512, T]
    memN = np.zeros((B, NC_CH * 128, MEM_DIM), BF16)
    memN[:, :T] = mem_bf

    win = np.zeros((B, NWIN, T), np.float32)
    awc = np.asarray(attention_weights_cat)
    for k in range(KERNEL):
        sh = k - (KERNEL // 2)
        lo, hi = max(0, -sh), min(T, T - sh)
        win[:, 2 * k, lo:hi] = awc[:, 0, lo + sh:hi + sh]
        win[:, 2 * k + 1, lo:hi] = awc[:, 1, lo + sh:hi + sh]
    win2 = win.astype(BF16)

    wc2 = np.ascontiguousarray(W_conv.reshape(NWIN, N_FILTERS)).astype(BF16)

    hTd = np.ascontiguousarray(attention_hidden_state.T)     # [1024, B]
    per_core = []
    for i in range(N_CORES):
        bs = slice(i * BL, (i + 1) * BL)
        per_core.append({
            "memT": np.ascontiguousarray(memT[bs]),
            "memN": np.ascontiguousarray(memN[bs]),
            "win": np.ascontiguousarray(win2[bs]),
            "hT": np.ascontiguousarray(
                hTd[:, bs].reshape(Q_DIM // 128, 128, BL)),
            "wq": np.ascontiguousarray(
                W_query.reshape(Q_DIM // 128, 128, ATT_DIM)).astype(np.float32),
            "wm": np.ascontiguousarray(
                W_memory.reshape(MEM_DIM // 128, 128, ATT_DIM)).astype(BF16),
            "wconv": wc2,
            "wloc": np.ascontiguousarray(W_loc).astype(BF16),
            "wv": np.ascontiguousarray(W_v).astype(BF16),
        })
    return per_core


def kernel(attention_hidden_state, memory, attention_weights_cat,
           W_query, W_memory, W_v, W_conv, W_loc):
    from concourse.bass_utils import run_bass_kernel_spmd

    args = dict(
        attention_hidden_state=np.asarray(attention_hidden_state, np.float32),
        memory=np.asarray(memory, np.float32),
        attention_weights_cat=np.asarray(attention_weights_cat, np.float32),
        W_query=np.asarray(W_query, np.float32),
        W_memory=np.asarray(W_memory, np.float32),
        W_v=np.asarray(W_v, np.float32),
        W_conv=np.asarray(W_conv, np.float32),
        W_loc=np.asarray(W_loc, np.float32),
    )
    in_maps = _host_prep(**args)
    nc = _build_bass()
    res = run_bass_kernel_spmd(nc, in_maps, core_ids=list(range(N_CORES)))
    ctx = np.concatenate([r["out_ctx"] for r in res.results], axis=0)
    w = np.concatenate([r["out_w"] for r in res.results], axis=0)
    return ctx, w
